# revision 17
# baseline (speedup 1.0000x reference)
"""Trainium2 Bass kernel for nn_ContextEncodingModule (vq_codebook).

Reference computation (per sample b):
  feats = inputs[b].reshape(N=H*W, F)
  sqnorms[n,k] = ||x_n||^2 - 2 x_n.c_k + ||c_k||^2
  s = softmax(sqnorms * smoothing, axis=k)
  enc = s^T feats - s_sum * codewords            (K, F)
  e = sum_k relu(bn(enc))                        (F,)
  attn = sigmoid(e @ fc_enc_w + fc_enc_b)        (F,)
  featuremaps_out[b] = attn * inputs[b]
  se_loss_out[b] = e @ fc_se_w + fc_se_b

Sharding: pure data parallel over batch, B=16 -> 2 samples per core on 8 cores.
All parameters replicated; no collectives.

Per-core dataflow (per (128, 512) tile of feats):
  load f32 -> cast bf16 -> DMA-xbar transpose (featsT bf16) ->
  PE: z = -2*sm*x.C^T + sm*c2 (bf16 matmuls, f32 accum) ->
  DVE: z += x2 * smoothing ; ACT: u = exp(z), r = rowsum(u) ->
  DVE: s = u / r -> PE: enc += s^T x (f32), s_sum += s^T 1
Sample tail: correction, BN+relu, column sums, attn/se matmuls (f32),
sigmoid via exp + reciprocal, attn broadcast via PE outer product.
Scale pass on GpSimd (frees DVE/ACT), store.
"""

import numpy as np
import ml_dtypes
from contextlib import ExitStack

B, H, W_, F, K = 16, 64, 64, 512, 32
NCORES = 8
S = B // NCORES          # samples per core
N = H * W_               # 4096 rows per sample
P = 128
T = N // P               # 32 tiles per sample
TPB = 8                  # tiles per transpose block
NBLK = T // TPB
BN_EPS = 1e-3

_CACHE = {}


def _legalize_sync_waits(nc, mybir):
    """This walrus build encodes at most 1-2 sem waits per instruction
    (1 for CTRL_NO/drain and SWDGE DMA). Move excess waits emitted by the
    Tile scheduler onto same-engine NoOp carriers inserted just before the
    over-limit instruction (same-engine program order preserves semantics)."""
    n = 0
    for func in nc.m.functions:
        for blk in func.blocks:
            li = list(blk.instructions)
            out = []
            changed = False
            for inst in li:
                si = getattr(inst, 'sync_info', None)
                waits = list(si.on_wait) if si is not None else []
                t = type(inst).__name__
                eng = inst.engine
                limit = 1
                if len(waits) > limit:
                    changed = True
                    excess, keep = waits[:-limit], waits[-limit:]
                    for w in excess:
                        n += 1
                        nop = mybir.InstNoOp(
                            name=f"zz_waitnop_{n}", engine=eng,
                            sync_info=mybir.SyncInfo(on_wait=[w], on_update=[]))
                        nc.register_instruction(nop)
                        out.append(nop)
                    inst.sync_info = mybir.SyncInfo(
                        on_wait=keep, on_update=list(si.on_update))
                out.append(inst)
            if changed:
                blk.instructions = out
    return n


def _build_bass():
    import concourse.bass as bass
    import concourse.tile as tile
    from concourse import mybir

    dt = mybir.dt
    f32, bf = dt.float32, dt.bfloat16
    AF = mybir.ActivationFunctionType
    OP = mybir.AluOpType

    nc = bass.Bass("TRN2", target_bir_lowering=False, debug=False)

    x_d = nc.dram_tensor("x", [S * N, F], dt.float32r, kind="ExternalInput").ap()
    cwts_d = nc.dram_tensor("cwts", [F, K], bf, kind="ExternalInput").ap()
    c2s_d = nc.dram_tensor("c2s", [1, K], bf, kind="ExternalInput").ap()
    sm_d = nc.dram_tensor("sm", [1, K], f32, kind="ExternalInput").ap()
    cw_d = nc.dram_tensor("cw", [K, F], f32, kind="ExternalInput").ap()
    g2_d = nc.dram_tensor("g2", [1, F], f32, kind="ExternalInput").ap()
    b2_d = nc.dram_tensor("b2", [1, F], f32, kind="ExternalInput").ap()
    wfc_d = nc.dram_tensor("wfc", [F, F], f32, kind="ExternalInput").ap()
    bfc_d = nc.dram_tensor("bfc", [1, F], f32, kind="ExternalInput").ap()
    wse_d = nc.dram_tensor("wse", [F, 1], f32, kind="ExternalInput").ap()
    bse_d = nc.dram_tensor("bse", [1, 1], f32, kind="ExternalInput").ap()
    non_d = nc.dram_tensor("non", [P, 1], dt.float32r, kind="ExternalInput").ap()
    fm_d = nc.dram_tensor("fm", [S * N, F], f32, kind="ExternalOutput").ap()
    se_d = nc.dram_tensor("se", [S, 1], f32, kind="ExternalOutput").ap()

    f32r = dt.float32r

    with tile.TileContext(nc) as tc, ExitStack() as ctx:
        consts = ctx.enter_context(tc.tile_pool(name="consts", bufs=1))
        xpool = ctx.enter_context(tc.tile_pool(name="xpool", bufs=44))
        bfpool = ctx.enter_context(tc.tile_pool(name="bfpool", bufs=6))
        sqpool = ctx.enter_context(tc.tile_pool(name="sqpool", bufs=3))
        ftpool = ctx.enter_context(tc.tile_pool(name="ftpool", bufs=4))
        colpool = ctx.enter_context(tc.tile_pool(name="colpool", bufs=10))
        upool = ctx.enter_context(tc.tile_pool(name="upool", bufs=4))
        spool = ctx.enter_context(tc.tile_pool(name="spool", bufs=4))
        tailpool = ctx.enter_context(tc.tile_pool(name="tailpool", bufs=2))
        abpool = ctx.enter_context(tc.tile_pool(name="abpool", bufs=2))
        sepool = ctx.enter_context(tc.tile_pool(name="sepool", bufs=1))
        opool = ctx.enter_context(tc.tile_pool(name="opool", bufs=6))
        ftps = ctx.enter_context(tc.tile_pool(name="ftps", bufs=2, space="PSUM"))
        zps = ctx.enter_context(tc.tile_pool(name="zps", bufs=2, space="PSUM"))
        encps = ctx.enter_context(tc.tile_pool(name="encps", bufs=1, space="PSUM"))
        ssumps = ctx.enter_context(tc.tile_pool(name="ssumps", bufs=1, space="PSUM"))
        smallps = ctx.enter_context(tc.tile_pool(name="smallps", bufs=2, space="PSUM"))

        # ---- constants ----
        cwts_sb = consts.tile([P, 4, K], bf, tag="cwts")
        nc.sync.dma_start(cwts_sb, cwts_d.rearrange("(c p) k -> p c k", p=P))
        c2s_sb = consts.tile([1, K], bf, tag="c2s")
        nc.sync.dma_start(c2s_sb, c2s_d)
        sm_bc = consts.tile([P, K], f32, tag="smbc")
        nc.gpsimd.dma_start(sm_bc, sm_d.to_broadcast([P, K]))
        cw_sb = consts.tile([K, F], f32, tag="cw")
        nc.sync.dma_start(cw_sb, cw_d)
        g2_bc = consts.tile([K, F], f32, tag="g2")
        nc.gpsimd.dma_start(g2_bc, g2_d.to_broadcast([K, F]))
        b2_bc = consts.tile([K, F], f32, tag="b2")
        nc.gpsimd.dma_start(b2_bc, b2_d.to_broadcast([K, F]))
        wfc_sb = consts.tile([P, 4, F], f32, tag="wfc")
        nc.sync.dma_start(wfc_sb, wfc_d.rearrange("(c p) f -> p c f", p=P))
        bfc_sb = consts.tile([1, F], f32, tag="bfc")
        nc.sync.dma_start(bfc_sb, bfc_d)
        wse_sb = consts.tile([P, 4], f32, tag="wse")
        nc.sync.dma_start(wse_sb, wse_d.rearrange("(c p) one -> p (c one)", p=P))
        bse_sb = consts.tile([1, 1], f32, tag="bse")
        nc.sync.dma_start(bse_sb, bse_d)

        ones_bf = consts.tile([1, P], bf, tag="ones_bf")
        nc.vector.memset(ones_bf, 1.0)
        ident_bf = consts.tile([P, P], bf, tag="ident")
        from concourse.masks import make_identity
        make_identity(nc, ident_bf)
        nones_f32 = consts.tile([P, 1], f32r, tag="nones")
        nc.sync.dma_start(nones_f32, non_d)
        ones_k = consts.tile([K, 1], f32, tag="ones_k")
        nc.vector.memset(ones_k, 1.0)
        ones_1p = consts.tile([1, P], f32, tag="ones_1p")
        nc.vector.memset(ones_1p, 1.0)

        se_acc = sepool.tile([1, S], f32, tag="seacc")

        for s in range(S):
            x_tiles = []
            enc_ps = encps.tile([K, F], f32, tag="enc")
            ssum_ps = ssumps.tile([K, 1], f32, tag="ssum")
            ftT = None
            for j in range(T):
                jj = j % TPB
                row0 = (s * T + j) * P

                x_t = xpool.tile([P, F], f32r, tag="x")
                x_tf = x_t.bitcast(f32)
                nc.sync.dma_start(x_t, x_d[row0:row0 + P, :])
                x_tiles.append(x_t)

                xb_t = bfpool.tile([P, F], bf, tag="xb")
                nc.vector.tensor_copy(xb_t, x_tf)

                sq_t = sqpool.tile([P, F], bf, tag="sq")
                x2_t = colpool.tile([P, 1], f32, tag="x2")
                nc.scalar.activation(sq_t, x_tf, AF.Square, accum_out=x2_t)

                ft_ps = ftps.tile([P, 4, P], bf, tag="ftp")
                for c in range(4):
                    nc.tensor.transpose(
                        ft_ps[:, c, :], xb_t[:, c * P:(c + 1) * P], ident_bf)
                ft_sb = ftpool.tile([P, 4, P], bf, tag="ft")
                nc.vector.tensor_copy(ft_sb, ft_ps)

                z_ps = zps.tile([P, K], f32, tag="z")
                for c in range(4):
                    nc.tensor.matmul(
                        z_ps, ft_sb[:, c, :],
                        cwts_sb[:, c, :], start=(c == 0), stop=False,
                        skip_group_check=True)
                nc.tensor.matmul(z_ps, ones_bf, c2s_sb, start=False,
                                 stop=True, skip_group_check=True)

                # z += x2 * smoothing (broadcast over k)
                nc.vector.scalar_tensor_tensor(
                    z_ps, sm_bc, x2_t, z_ps, op0=OP.mult, op1=OP.add)

                u_t = upool.tile([P, K], f32, tag="u")
                r_t = colpool.tile([P, 1], f32, tag="r")
                nc.scalar.activation(u_t, z_ps, AF.Exp, accum_out=r_t)

                rinv_t = colpool.tile([P, 1], f32, tag="rinv")
                nc.vector.reciprocal(rinv_t, r_t)
                s_t = spool.tile([P, K], f32r, tag="s")
                nc.vector.tensor_scalar_mul(s_t, u_t, rinv_t)

                nc.tensor.matmul(enc_ps, s_t, x_t,
                                 start=(j == 0), stop=(j == T - 1),
                                 skip_group_check=True)
                nc.tensor.matmul(ssum_ps, s_t.bitcast(f32),
                                 nones_f32.bitcast(f32), start=(j == 0),
                                 stop=(j == T - 1), skip_group_check=True)

            # ---- per-sample tail ----
            enc_sb = tailpool.tile([K, F], f32, tag="enc_sb")
            nc.vector.scalar_tensor_tensor(
                enc_sb, cw_sb, ssum_ps[:, 0:1], enc_ps,
                op0=OP.mult, op1=OP.add)
            # BN (folded) + relu
            nc.vector.tensor_mul(enc_sb, enc_sb, g2_bc)
            nc.vector.tensor_add(enc_sb, enc_sb, b2_bc)
            nc.vector.tensor_scalar_max(enc_sb, enc_sb, 0.0)

            eT_ps = smallps.tile([P, 4], f32, tag="small")
            for c in range(4):
                nc.tensor.matmul(eT_ps[:, c:c + 1],
                                 enc_sb[:, c * P:(c + 1) * P], ones_k,
                                 start=True, stop=True, skip_group_check=True)
            eT_sb = tailpool.tile([P, 4], f32, tag="eT")
            nc.vector.tensor_copy(eT_sb, eT_ps)

            attn_ps = smallps.tile([1, F], f32, tag="small")
            for c in range(4):
                nc.tensor.matmul(attn_ps, eT_sb[:, c:c + 1],
                                 wfc_sb[:, c, :], start=(c == 0),
                                 stop=(c == 3), skip_group_check=True)
            # sigmoid(raw + b) = 1 / (1 + exp(-(raw + b)))  (stay in exp table set)
            attn_sb = tailpool.tile([1, F], f32, tag="attn")
            nc.vector.tensor_add(attn_sb, attn_ps, bfc_sb)
            nc.scalar.activation(attn_sb, attn_sb, AF.Exp, scale=-1.0)
            nc.vector.tensor_scalar_add(attn_sb, attn_sb, 1.0)
            nc.vector.reciprocal(attn_sb, attn_sb)

            ab_ps = smallps.tile([P, F], f32, tag="small")
            nc.tensor.matmul(ab_ps, ones_1p, attn_sb, start=True, stop=True,
                             skip_group_check=True)
            attn_bc = abpool.tile([P, F], f32, tag="attn_bc")
            nc.vector.tensor_copy(attn_bc, ab_ps)

            se_ps = smallps.tile([1, 1], f32, tag="small")
            for c in range(4):
                nc.tensor.matmul(se_ps, eT_sb[:, c:c + 1],
                                 wse_sb[:, c:c + 1], start=(c == 0),
                                 stop=(c == 3), skip_group_check=True)
            nc.scalar.activation(se_acc[:, s:s + 1], se_ps, AF.Identity,
                                 bias=bse_sb[:, 0:1])

            # ---- scale + store pass (GpSimd keeps DVE/ACT free) ----
            for j in range(T):
                row0 = (s * T + j) * P
                x_t = x_tiles[j]
                o_t = opool.tile([P, F], f32, tag="o", name=f"o_{s}_{j}")
                nc.gpsimd.tensor_mul(o_t, x_t.bitcast(f32), attn_bc)
                nc.sync.dma_start(fm_d[row0:row0 + P, :], o_t)
            x_tiles = None

        nc.sync.dma_start(se_d, se_acc)

    _legalize_sync_waits(nc, mybir)
    return nc


def _get_nc():
    if "nc" not in _CACHE:
        _CACHE["nc"] = _build_bass()
    return _CACHE["nc"]


def kernel(inputs, codewords, smoothing, bn_gamma, bn_beta, bn_mean, bn_var,
           fc_enc_w, fc_enc_b, fc_se_w, fc_se_b):
    from concourse.bass_utils import run_bass_kernel_spmd

    bf16 = ml_dtypes.bfloat16
    x = np.ascontiguousarray(np.asarray(inputs, dtype=np.float32))
    C = np.asarray(codewords, dtype=np.float32)
    sm = np.asarray(smoothing, dtype=np.float32)
    g = np.asarray(bn_gamma, dtype=np.float64)
    be = np.asarray(bn_beta, dtype=np.float64)
    mu = np.asarray(bn_mean, dtype=np.float64)
    var = np.asarray(bn_var, dtype=np.float64)

    cwts = np.ascontiguousarray(
        (-2.0 * sm[:, None].astype(np.float64) * C.astype(np.float64)).T
    ).astype(bf16)                                             # (F, K)
    c2s = (sm.astype(np.float64)
           * (C.astype(np.float64) ** 2).sum(1)).astype(np.float32)
    c2s = c2s.reshape(1, K).astype(bf16)
    g2 = (g / np.sqrt(var + BN_EPS)).astype(np.float32).reshape(1, F)
    b2 = (be - mu * (g / np.sqrt(var + BN_EPS))).astype(np.float32).reshape(1, F)

    common = {
        "cwts": cwts,
        "c2s": c2s,
        "sm": sm.reshape(1, K).astype(np.float32),
        "cw": C,
        "g2": g2,
        "b2": b2,
        "wfc": np.ascontiguousarray(np.asarray(fc_enc_w, dtype=np.float32)),
        "bfc": np.asarray(fc_enc_b, dtype=np.float32).reshape(1, F),
        "wse": np.ascontiguousarray(np.asarray(fc_se_w, dtype=np.float32)),
        "bse": np.asarray(fc_se_b, dtype=np.float32).reshape(1, 1),
        "non": np.full((P, 1), -1.0, dtype=np.float32),
    }
    x_flat = x.reshape(B, N, F)
    in_maps = []
    for i in range(NCORES):
        m = dict(common)
        m["x"] = np.ascontiguousarray(
            x_flat[i * S:(i + 1) * S].reshape(S * N, F))
        in_maps.append(m)

    nc = _get_nc()
    res = run_bass_kernel_spmd(nc, in_maps, core_ids=list(range(NCORES)))
    _CACHE["last_result"] = res
    fm = np.concatenate([r["fm"] for r in res.results], axis=0)
    fm = fm.reshape(B, H, W_, F).astype(np.float32)
    se = np.concatenate([r["se"] for r in res.results], axis=0)
    se = se.reshape(B, 1).astype(np.float32)
    return fm, se
